# revision 33
# baseline (speedup 1.0000x reference)
"""Trainium2 Bass kernel for nn_Encoder_LSTM (4x LSTMCell with zero state over
packed ragged tokens).

Math (from the reference): all rows independent; for each output row j with
source row s(j) (the ragged gather), and each of 4 layers:
    gates = x @ W_ih^T + (b_ih + b_hh);  i, f, g, o = split(gates)
    c = sigmoid(i) * tanh(g);  h = sigmoid(o) * tanh(c)      (f is unused)
Outputs: (output=h4, h1, c1, h2, c2, h3, c3, h4, c4), each [sum(bs), 512] fp32.

Strategy (v2):
  - The ragged gather packed_x[s] re-reads source rows; only U = max(s)+1
    distinct rows exist (all referenced).  The device computes each distinct
    row's h/c once; the host applies the exact same gather D[s] the reference
    uses (pure row indexing, no math) to expand duplicates.
  - Shard distinct rows in contiguous blocks of U/8 across the 8 cores.
  - Per core: 17 tiles of 128 rows.  x arrives pre-transposed (feat-major
    chunks) so layer 1 needs no on-device transpose; h1..h3 are cast to bf16
    and PE-transposed between layers.
  - Gates are packed [i, o, g] * 512 (f unused).  Per tile-layer: 12 bf16
    matmuls (K=4 chunks x N=3x512) accumulate into PSUM; DVE adds the
    (free-axis) bias; ACT applies sigmoid/tanh; Pool (gpsimd) and DVE share
    the elementwise muls and the f32->bf16 cast so no engine exceeds the PE's
    per-tile budget.
  - Tiles are processed in software-pipelined groups of 4, stage-batched, so
    the PE always has another tile's matmuls to run while a tile's activation
    chain is in flight (keeps the PE HAM clock at 2.4 GHz).
  - All h/c of one tile live in one [128, 8*512] f32 SBUF tile, stored with a
    single contiguous 2 MB DMA per tile.
"""

import sys

if "/opt/trn_rl_repo" not in sys.path:
    sys.path.insert(0, "/opt/trn_rl_repo")

import numpy as np
import ml_dtypes

P = 128
H = 512
G = 1536          # 3 packed gates [i, o, g] * 512
J = 8             # fused outputs [h1, c1, h2, c2, h3, c3, h4, c4]
NCORES = 8
D_PIPE = 4        # tiles interleaved in the software pipeline
OUT_NAMES = ["h1", "c1", "h2", "c2", "h3", "c3", "h4", "c4"]


# ---------------------------------------------------------------- host plan

def _make_plan(batch_sizes):
    bs = np.asarray(batch_sizes).astype(np.int64)
    s = np.concatenate([i * b + np.arange(b) for i, b in enumerate(bs)]).astype(np.int64)
    Nout = int(s.size)
    U = int(s.max()) + 1
    # contiguous block shard: core c owns distinct rows [starts[c], starts[c+1])
    base, rem = divmod(U, NCORES)
    counts = [base + (1 if c < rem else 0) for c in range(NCORES)]
    starts = np.concatenate([[0], np.cumsum(counts)])
    T_tiles = max((n + P - 1) // P for n in counts)
    return dict(s=s, Nout=Nout, U=U, counts=counts, starts=starts, T_tiles=T_tiles)


def _pack_weights(inputs):
    """-> w [P, 16*G] bf16 (W^T chunks, [li,k] block at cols (4li+k)*G),
          b [P, 4*G] f32 (bias broadcast across partitions)."""
    w = np.zeros((P, 16 * G), ml_dtypes.bfloat16)
    b = np.zeros((P, 4 * G), np.float32)
    for li in range(4):
        W = np.asarray(inputs[f"W_ih{li+1}"], np.float32)        # [2048, 512]
        bb = (np.asarray(inputs[f"b_ih{li+1}"], np.float32)
              + np.asarray(inputs[f"b_hh{li+1}"], np.float32))   # [2048]
        Wigo = np.concatenate([W[0:512], W[1536:2048], W[1024:1536]], axis=0)
        bigo = np.concatenate([bb[0:512], bb[1536:2048], bb[1024:1536]])
        WT = Wigo.T.astype(ml_dtypes.bfloat16)                   # [512, 1536]
        for k in range(4):
            w[:, (li * 4 + k) * G:(li * 4 + k + 1) * G] = WT[k * P:(k + 1) * P]
        b[:, li * G:(li + 1) * G] = bigo[None, :]
    return w, b


def _pack_x(x_rows, T_tiles):
    """x_rows [n, 512] f32 -> pre-transposed bf16 [128, T*4*128]:
    col block (4t+k) holds x[t*128:(t+1)*128, k*128:(k+1)*128].T"""
    n = x_rows.shape[0]
    xp = np.zeros((T_tiles * P, H), np.float32)
    xp[:n] = x_rows
    y = xp.reshape(T_tiles, P, 4, P).transpose(3, 0, 2, 1)   # [p, t, k, j]
    return np.ascontiguousarray(y.reshape(P, T_tiles * H)).astype(ml_dtypes.bfloat16)


# ---------------------------------------------------------------- bass build

def _build_nc(T_tiles):
    import concourse.mybir as mybir
    from concourse import bacc
    from concourse.masks import make_identity
    from concourse.tile import TileContext

    dt = mybir.dt
    AF = mybir.ActivationFunctionType

    nc = bacc.Bacc()
    x_d = nc.dram_tensor("x", [P, T_tiles * H], dt.bfloat16, kind="ExternalInput")
    w_d = nc.dram_tensor("w", [P, 16 * G], dt.bfloat16, kind="ExternalInput")
    b_d = nc.dram_tensor("b", [P, 4 * G], dt.float32, kind="ExternalInput")
    o_d = nc.dram_tensor("hc", [T_tiles * P, J * H], dt.float32, kind="ExternalOutput")

    # Multi-group layer ping-pong: tiles go in groups of 2; 2-3 groups rotate
    # layer-stages (A,L1)(B,L1)(A,L2)(B,L2)... so each tile's h has >=1 full
    # stage of slack before its transpose feeds the next layer's matmul.
    pairs = [list(range(i, min(i + 2, T_tiles))) for i in range(0, T_tiles, 2)]
    stages = []   # (tiles, li)
    i = 0
    while i < len(pairs):
        nblk = min(3, len(pairs) - i)
        blk = sorted(pairs[i:i + nblk], key=len)   # smallest group first so
        for li in range(4):                        # the last stages overlap
            for grp in blk:
                stages.append((grp, li))
        i += nblk

    with TileContext(nc) as tc:
        with (
            tc.tile_pool(name="const", bufs=1) as constp,
            tc.tile_pool(name="aT", bufs=6) as aTp,
            tc.tile_pool(name="gsb", bufs=4) as gsbp,
            tc.tile_pool(name="sio", bufs=6) as siop,
            tc.tile_pool(name="tg", bufs=6) as tgp,
            tc.tile_pool(name="tc_", bufs=6) as tcp,
            tc.tile_pool(name="h16", bufs=6) as h16p,
            tc.tile_pool(name="hc", bufs=8) as hcp,
            tc.tile_pool(name="psg", bufs=2, space="PSUM") as psgp,
            tc.tile_pool(name="pst", bufs=2, space="PSUM") as pstp,
        ):
            # load order: only what the first stages need, then the rest, so
            # layer-1 matmuls start ~13us in instead of 45
            x_all = constp.tile([P, T_tiles * H], dt.bfloat16)
            x_head = min(6 * H, T_tiles * H)
            nc.sync.dma_start(x_all[:, 0:x_head], x_d[:, 0:x_head])
            w_li = [constp.tile([P, 4 * G], dt.bfloat16, name=f"w{li}")
                    for li in range(4)]
            b_li = [constp.tile([P, G], dt.float32, name=f"b{li}")
                    for li in range(4)]
            nc.sync.dma_start(w_li[0][:], w_d[:, 0:4 * G])
            nc.sync.dma_start(b_li[0][:], b_d[:, 0:G])
            if x_head < T_tiles * H:
                nc.sync.dma_start(x_all[:, x_head:], x_d[:, x_head:])
            for li in range(1, 4):
                nc.sync.dma_start(w_li[li][:],
                                  w_d[:, li * 4 * G:(li + 1) * 4 * G])
                nc.sync.dma_start(b_li[li][:], b_d[:, li * G:(li + 1) * G])
            id_bf = constp.tile([P, P], dt.bfloat16)
            make_identity(nc, id_bf[:])

            hc_t = {}     # (t, li) -> hcli tile (holds h | c, f32)
            h16_t = {}    # (t, li) -> bf16 copy of h(t, li)
            aT_t = {}     # (t, li) -> bf16 transposed activation for layer li

            def emit_trcb(t, li):
                # PE: transpose bf16 h(t, li-1) (1cyc/col) into PSUM, then
                # DVE: copy PSUM->SBUF bf16 lhsT for layer li.
                h_prev = h16_t.pop((t, li - 1))
                pt = pstp.tile([P, H], dt.bfloat16, tag="pst",
                               name=f"ptr_{t}_{li}")
                for k in range(4):
                    nc.tensor.transpose(pt[:, k * P:(k + 1) * P],
                                        h_prev[:, k * P:(k + 1) * P],
                                        id_bf[:])
                aTs = aTp.tile([P, H], dt.bfloat16, tag="aT",
                               name=f"aT_{t}_{li}")
                nc.vector.tensor_copy(aTs[:], pt[:])
                aT_t[(t, li)] = aTs

            for j, (ts, li) in enumerate(stages):
                # transposes for the NEXT stage are interleaved between this
                # stage's matmuls (their h inputs are one full stage old)
                nxt = None
                if j + 1 < len(stages) and stages[j + 1][1] > 0:
                    nxt = stages[j + 1]
                sio = {}
                tgd = {}
                tc_ = {}
                # phase 1: matmuls + next-stage transpose prep + bias adds +
                # sigmoids/tanh(g).  DVE sees adds first (nothing blocks them)
                for idx, t in enumerate(ts):
                    aT = x_all[:, t * H:(t + 1) * H] if li == 0 \
                        else aT_t.pop((t, li))[:]
                    gp = psgp.tile([P, G], dt.float32, tag="psg",
                                   name=f"gps_{t}_{li}")
                    for k in range(4):
                        lhsT = aT[:, k * P:(k + 1) * P]
                        for n in range(3):
                            nc.tensor.matmul(
                                gp[:, n * H:(n + 1) * H],
                                lhsT,
                                w_li[li][:, k * G + n * H:k * G + (n + 1) * H],
                                start=(k == 0),
                                stop=(k == 3),
                            )
                    if nxt is not None and idx < len(nxt[0]):
                        emit_trcb(nxt[0][idx], nxt[1])
                    gs = gsbp.tile([P, G], dt.float32, tag="gsb",
                                   name=f"gsb_{t}_{li}")
                    nc.vector.tensor_add(gs[:], gp[:], b_li[li][:])
                    so = siop.tile([P, 1024], dt.float32, tag="sio",
                                   name=f"sio_{t}_{li}")
                    sio[t] = so
                    nc.scalar.activation(so[:], gs[:, 0:1024], AF.Sigmoid)
                    tgt = tgp.tile([P, H], dt.float32, tag="tg",
                                   name=f"tg_{t}_{li}")
                    tgd[t] = tgt
                    nc.scalar.activation(tgt[:], gs[:, 1024:G], AF.Tanh)
                if nxt is not None:
                    for idx in range(len(ts), len(nxt[0])):
                        emit_trcb(nxt[0][idx], nxt[1])
                # phase 2: c = sig(i)*tanh(g) (DVE in L1 where it has slack,
                # Pool otherwise), then tanh(c) on ACT
                for t in ts:
                    hct = hcp.tile([P, 2 * H], dt.float32, tag="hc",
                                   name=f"hc_{t}_{li}")
                    hc_t[(t, li)] = hct
                    mul_eng = nc.vector if li == 0 else nc.gpsimd
                    mul_eng.tensor_mul(hct[:, H:2 * H], sio[t][:, 0:H],
                                       tgd[t][:])
                for t in ts:
                    tct = tcp.tile([P, H], dt.float32, tag="tc",
                                   name=f"tc_{t}_{li}")
                    tc_[t] = tct
                    nc.scalar.activation(tct[:], hc_t[(t, li)][:, H:2 * H],
                                         AF.Tanh)
                # phase 3: h = sig(o)*tanh(c) on Pool, then store; DVE casts
                # h to bf16 for the next layer's transpose (after the adds,
                # so it never blocks PSUM drainage)
                for t in ts:
                    nc.gpsimd.tensor_mul(hc_t[(t, li)][:, 0:H],
                                         sio[t][:, H:1024], tc_[t][:])
                    nc.sync.dma_start(
                        o_d[t * P:(t + 1) * P, li * 2 * H:(li + 1) * 2 * H],
                        hc_t[(t, li)][:])
                    if li < 3:
                        h16 = h16p.tile([P, H], dt.bfloat16, tag="h16",
                                        name=f"h16_{t}_{li}")
                        h16_t[(t, li)] = h16
                        nc.vector.tensor_copy(h16[:], hc_t[(t, li)][:, 0:H])

    nc.compile()
    return nc


# ---------------------------------------------------------------- entry point

def _ensure_axon_hooks():
    """bass_utils' trace path imports antenv.axon_hooks, which some images
    lack; install a shim that drives NTFF profiling via libaxon_pjrt.so
    (mirrors the boot-side _ntff_profile_via_ctypes) or degrades to None."""
    try:
        import antenv.axon_hooks  # noqa: F401
        return
    except ImportError:
        pass
    import types
    import contextlib
    import ctypes

    def _build_hook():
        so = "/opt/axon/libaxon_pjrt.so"
        try:
            lib = ctypes.CDLL(so)
        except OSError:
            return None
        if not hasattr(lib, "axon_start_nrt_profile"):
            return None
        lib.axon_start_nrt_profile.argtypes = [
            ctypes.POINTER(ctypes.c_int64), ctypes.c_size_t]
        lib.axon_start_nrt_profile.restype = ctypes.c_int64
        lib.axon_stop_nrt_profile.argtypes = [ctypes.c_char_p]
        lib.axon_stop_nrt_profile.restype = ctypes.c_int64

        @contextlib.contextmanager
        def _hook(output_dir, device_ids):
            import jax
            jax.devices()
            if device_ids:
                ids = (ctypes.c_int64 * len(device_ids))(*device_ids)
                rc = lib.axon_start_nrt_profile(ids, len(device_ids))
            else:
                rc = lib.axon_start_nrt_profile(None, 0)
            if rc != 0:
                raise RuntimeError(f"axon_start_nrt_profile rc={rc}")
            try:
                yield
            finally:
                n = lib.axon_stop_nrt_profile(str(output_dir).encode())
                print(f"ntff profile: {n} file(s) written to {output_dir}",
                      file=sys.stderr)

        return _hook

    box = [None, False]

    def set_axon_ntff_profile_hook(h):
        box[0] = h
        box[1] = True

    def get_axon_ntff_profile_hook():
        if not box[1]:
            box[0] = _build_hook()
            box[1] = True
        return box[0]

    mod = types.ModuleType("antenv.axon_hooks")
    mod.set_axon_ntff_profile_hook = set_axon_ntff_profile_hook
    mod.get_axon_ntff_profile_hook = get_axon_ntff_profile_hook
    import antenv
    sys.modules["antenv.axon_hooks"] = mod
    antenv.axon_hooks = mod


_cache = {}


def kernel(**inputs):
    packed_x = np.asarray(inputs["packed_x"], np.float32)
    bs = np.asarray(inputs["batch_sizes"])

    key = bs.tobytes()
    if key not in _cache:
        plan = _make_plan(bs)
        nc = _build_nc(plan["T_tiles"])
        _cache[key] = (plan, nc)
    plan, nc = _cache[key]

    w, b = _pack_weights(inputs)
    T_tiles = plan["T_tiles"]
    starts = plan["starts"]

    in_maps = []
    for c in range(NCORES):
        xr = packed_x[starts[c]:starts[c + 1]]
        in_maps.append({"x": _pack_x(xr, T_tiles), "w": w, "b": b})

    from concourse.bass_utils import run_bass_kernel_spmd
    _ensure_axon_hooks()
    res = run_bass_kernel_spmd(nc, in_maps, core_ids=list(range(NCORES)))
    global last_result
    last_result = res

    # Reassemble: distinct-row tensor D [U, 8*512], then the reference's own
    # ragged gather D[s] expands duplicates (pure indexing, no math).
    s = plan["s"]
    Dfull = np.empty((plan["U"], J * H), np.float32)
    for c in range(NCORES):
        n = plan["counts"][c]
        Dfull[starts[c]:starts[c + 1]] = np.asarray(res.results[c]["hc"])[:n]
    gathered = Dfull[s]                                   # [Nout, 8*512]
    full = {nm: np.ascontiguousarray(gathered[:, jo * H:(jo + 1) * H])
            for jo, nm in enumerate(OUT_NAMES)}

    return (full["h4"], full["h1"], full["c1"], full["h2"], full["c2"],
            full["h3"], full["c3"], full["h4"], full["c4"])


if __name__ == "__main__":
    import reference
    inputs = reference.setup_inputs()
    out = kernel(**{k: np.asarray(v) for k, v in inputs.items()})
    print([o.shape for o in out])


# revision 34
# speedup vs baseline: 1.0120x; 1.0120x over previous
"""Trainium2 Bass kernel for nn_Encoder_LSTM (4x LSTMCell with zero state over
packed ragged tokens).

Math (from the reference): all rows independent; for each output row j with
source row s(j) (the ragged gather), and each of 4 layers:
    gates = x @ W_ih^T + (b_ih + b_hh);  i, f, g, o = split(gates)
    c = sigmoid(i) * tanh(g);  h = sigmoid(o) * tanh(c)      (f is unused)
Outputs: (output=h4, h1, c1, h2, c2, h3, c3, h4, c4), each [sum(bs), 512] fp32.

Strategy (v2):
  - The ragged gather packed_x[s] re-reads source rows; only U = max(s)+1
    distinct rows exist (all referenced).  The device computes each distinct
    row's h/c once; the host applies the exact same gather D[s] the reference
    uses (pure row indexing, no math) to expand duplicates.
  - Shard distinct rows in contiguous blocks of U/8 across the 8 cores.
  - Per core: 17 tiles of 128 rows.  x arrives pre-transposed (feat-major
    chunks) so layer 1 needs no on-device transpose; h1..h3 are cast to bf16
    and PE-transposed between layers.
  - Gates are packed [i, o, g] * 512 (f unused).  Per tile-layer: 12 bf16
    matmuls (K=4 chunks x N=3x512) accumulate into PSUM; DVE adds the
    (free-axis) bias; ACT applies sigmoid/tanh; Pool (gpsimd) and DVE share
    the elementwise muls and the f32->bf16 cast so no engine exceeds the PE's
    per-tile budget.
  - Tiles are processed in software-pipelined groups of 4, stage-batched, so
    the PE always has another tile's matmuls to run while a tile's activation
    chain is in flight (keeps the PE HAM clock at 2.4 GHz).
  - All h/c of one tile live in one [128, 8*512] f32 SBUF tile, stored with a
    single contiguous 2 MB DMA per tile.
"""

import sys

if "/opt/trn_rl_repo" not in sys.path:
    sys.path.insert(0, "/opt/trn_rl_repo")

import numpy as np
import ml_dtypes

P = 128
H = 512
G = 1536          # 3 packed gates [i, o, g] * 512
J = 8             # fused outputs [h1, c1, h2, c2, h3, c3, h4, c4]
NCORES = 8
D_PIPE = 4        # tiles interleaved in the software pipeline
OUT_NAMES = ["h1", "c1", "h2", "c2", "h3", "c3", "h4", "c4"]


# ---------------------------------------------------------------- host plan

def _make_plan(batch_sizes):
    bs = np.asarray(batch_sizes).astype(np.int64)
    s = np.concatenate([i * b + np.arange(b) for i, b in enumerate(bs)]).astype(np.int64)
    Nout = int(s.size)
    U = int(s.max()) + 1
    # contiguous block shard: core c owns distinct rows [starts[c], starts[c+1])
    base, rem = divmod(U, NCORES)
    counts = [base + (1 if c < rem else 0) for c in range(NCORES)]
    starts = np.concatenate([[0], np.cumsum(counts)])
    T_tiles = max((n + P - 1) // P for n in counts)
    return dict(s=s, Nout=Nout, U=U, counts=counts, starts=starts, T_tiles=T_tiles)


def _pack_weights(inputs):
    """-> w [P, 16*G] bf16 (W^T chunks, [li,k] block at cols (4li+k)*G),
          b [P, 4*G] f32 (bias broadcast across partitions)."""
    w = np.zeros((P, 16 * G), ml_dtypes.bfloat16)
    b = np.zeros((P, 4 * G), np.float32)
    for li in range(4):
        W = np.asarray(inputs[f"W_ih{li+1}"], np.float32)        # [2048, 512]
        bb = (np.asarray(inputs[f"b_ih{li+1}"], np.float32)
              + np.asarray(inputs[f"b_hh{li+1}"], np.float32))   # [2048]
        Wigo = np.concatenate([W[0:512], W[1536:2048], W[1024:1536]], axis=0)
        bigo = np.concatenate([bb[0:512], bb[1536:2048], bb[1024:1536]])
        WT = Wigo.T.astype(ml_dtypes.bfloat16)                   # [512, 1536]
        for k in range(4):
            w[:, (li * 4 + k) * G:(li * 4 + k + 1) * G] = WT[k * P:(k + 1) * P]
        b[:, li * G:(li + 1) * G] = bigo[None, :]
    return w, b


def _pack_x(x_rows, T_tiles):
    """x_rows [n, 512] f32 -> pre-transposed bf16 [128, T*4*128]:
    col block (4t+k) holds x[t*128:(t+1)*128, k*128:(k+1)*128].T"""
    n = x_rows.shape[0]
    xp = np.zeros((T_tiles * P, H), np.float32)
    xp[:n] = x_rows
    y = xp.reshape(T_tiles, P, 4, P).transpose(3, 0, 2, 1)   # [p, t, k, j]
    return np.ascontiguousarray(y.reshape(P, T_tiles * H)).astype(ml_dtypes.bfloat16)


# ---------------------------------------------------------------- bass build

def _build_nc(T_tiles):
    import concourse.mybir as mybir
    from concourse import bacc
    from concourse.masks import make_identity
    from concourse.tile import TileContext

    dt = mybir.dt
    AF = mybir.ActivationFunctionType

    nc = bacc.Bacc()
    x_d = nc.dram_tensor("x", [P, T_tiles * H], dt.bfloat16, kind="ExternalInput")
    w_d = nc.dram_tensor("w", [P, 16 * G], dt.bfloat16, kind="ExternalInput")
    b_d = nc.dram_tensor("b", [P, 4 * G], dt.float32, kind="ExternalInput")
    o_d = nc.dram_tensor("hc", [T_tiles * P, J * H], dt.float32, kind="ExternalOutput")

    # Multi-group layer ping-pong: tiles go in groups of 2; 2-3 groups rotate
    # layer-stages (A,L1)(B,L1)(A,L2)(B,L2)... so each tile's h has >=1 full
    # stage of slack before its transpose feeds the next layer's matmul.
    pairs = [list(range(i, min(i + 2, T_tiles))) for i in range(0, T_tiles, 2)]
    stages = []   # (tiles, li)
    i = 0
    while i < len(pairs):
        nblk = min(3, len(pairs) - i)
        blk = sorted(pairs[i:i + nblk], key=len)   # smallest group first so
        for li in range(4):                        # the last stages overlap
            for grp in blk:
                stages.append((grp, li))
        i += nblk

    with TileContext(nc) as tc:
        with (
            tc.tile_pool(name="const", bufs=1) as constp,
            tc.tile_pool(name="aT", bufs=6) as aTp,
            tc.tile_pool(name="gsb", bufs=4) as gsbp,
            tc.tile_pool(name="sio", bufs=6) as siop,
            tc.tile_pool(name="tg", bufs=6) as tgp,
            tc.tile_pool(name="tc_", bufs=6) as tcp,
            tc.tile_pool(name="h16", bufs=6) as h16p,
            tc.tile_pool(name="hc", bufs=8) as hcp,
            tc.tile_pool(name="psg", bufs=2, space="PSUM") as psgp,
            tc.tile_pool(name="pst", bufs=2, space="PSUM") as pstp,
        ):
            # load order: only what the first stages need, then the rest, so
            # layer-1 matmuls start ~13us in instead of 45
            x_all = constp.tile([P, T_tiles * H], dt.bfloat16)
            x_head = min(6 * H, T_tiles * H)
            nc.sync.dma_start(x_all[:, 0:x_head], x_d[:, 0:x_head])
            w_li = [constp.tile([P, 4 * G], dt.bfloat16, name=f"w{li}")
                    for li in range(4)]
            b_li = [constp.tile([P, G], dt.float32, name=f"b{li}")
                    for li in range(4)]
            nc.sync.dma_start(w_li[0][:], w_d[:, 0:4 * G])
            nc.sync.dma_start(b_li[0][:], b_d[:, 0:G])
            if x_head < T_tiles * H:
                nc.sync.dma_start(x_all[:, x_head:], x_d[:, x_head:])
            for li in range(1, 4):
                nc.sync.dma_start(w_li[li][:],
                                  w_d[:, li * 4 * G:(li + 1) * 4 * G])
                nc.sync.dma_start(b_li[li][:], b_d[:, li * G:(li + 1) * G])
            id_bf = constp.tile([P, P], dt.bfloat16)
            make_identity(nc, id_bf[:])

            hc_t = {}     # (t, li) -> hcli tile (holds h | c, f32)
            h16_t = {}    # (t, li) -> bf16 copy of h(t, li)
            aT_t = {}     # (t, li) -> bf16 transposed activation for layer li

            def emit_trcb(t, li):
                # PE: transpose bf16 h(t, li-1) (1cyc/col) into PSUM, then
                # DVE: copy PSUM->SBUF bf16 lhsT for layer li.
                h_prev = h16_t.pop((t, li - 1))
                pt = pstp.tile([P, H], dt.bfloat16, tag="pst",
                               name=f"ptr_{t}_{li}")
                for k in range(4):
                    nc.tensor.transpose(pt[:, k * P:(k + 1) * P],
                                        h_prev[:, k * P:(k + 1) * P],
                                        id_bf[:])
                aTs = aTp.tile([P, H], dt.bfloat16, tag="aT",
                               name=f"aT_{t}_{li}")
                nc.vector.tensor_copy(aTs[:], pt[:])
                aT_t[(t, li)] = aTs

            for j, (ts, li) in enumerate(stages):
                # transposes for the NEXT stage are interleaved between this
                # stage's matmuls (their h inputs are one full stage old)
                nxt = None
                if j + 1 < len(stages) and stages[j + 1][1] > 0:
                    nxt = stages[j + 1]
                sio = {}
                tgd = {}
                tc_ = {}
                # phase 1: matmuls + next-stage transpose prep + bias adds +
                # sigmoids/tanh(g).  DVE sees adds first (nothing blocks them)
                for idx, t in enumerate(ts):
                    aT = x_all[:, t * H:(t + 1) * H] if li == 0 \
                        else aT_t.pop((t, li))[:]
                    gp = psgp.tile([P, G], dt.float32, tag="psg",
                                   name=f"gps_{t}_{li}")
                    for k in range(4):
                        lhsT = aT[:, k * P:(k + 1) * P]
                        for n in range(3):
                            nc.tensor.matmul(
                                gp[:, n * H:(n + 1) * H],
                                lhsT,
                                w_li[li][:, k * G + n * H:k * G + (n + 1) * H],
                                start=(k == 0),
                                stop=(k == 3),
                            )
                    if nxt is not None and idx < len(nxt[0]):
                        emit_trcb(nxt[0][idx], nxt[1])
                    gs = gsbp.tile([P, G], dt.float32, tag="gsb",
                                   name=f"gsb_{t}_{li}")
                    nc.vector.tensor_add(gs[:], gp[:], b_li[li][:])
                    so = siop.tile([P, 1024], dt.float32, tag="sio",
                                   name=f"sio_{t}_{li}")
                    sio[t] = so
                    nc.scalar.activation(so[:], gs[:, 0:1024], AF.Sigmoid)
                    tgt = tgp.tile([P, H], dt.float32, tag="tg",
                                   name=f"tg_{t}_{li}")
                    tgd[t] = tgt
                    nc.scalar.activation(tgt[:], gs[:, 1024:G], AF.Tanh)
                if nxt is not None:
                    for idx in range(len(ts), len(nxt[0])):
                        emit_trcb(nxt[0][idx], nxt[1])
                # phase 2: c = sig(i)*tanh(g) (DVE in L1 where it has slack,
                # Pool otherwise), then tanh(c) on ACT
                for t in ts:
                    hct = hcp.tile([P, 2 * H], dt.float32, tag="hc",
                                   name=f"hc_{t}_{li}")
                    hc_t[(t, li)] = hct
                    mul_eng = nc.vector if li == 0 else nc.gpsimd
                    mul_eng.tensor_mul(hct[:, H:2 * H], sio[t][:, 0:H],
                                       tgd[t][:])
                for t in ts:
                    tct = tcp.tile([P, H], dt.float32, tag="tc",
                                   name=f"tc_{t}_{li}")
                    tc_[t] = tct
                    nc.scalar.activation(tct[:], hc_t[(t, li)][:, H:2 * H],
                                         AF.Tanh)
                # phase 3: h = sig(o)*tanh(c) on Pool (f32, for the store);
                # DVE computes the same product straight into bf16 for the
                # next layer's transpose -- cheaper than casting h afterwards
                # and independent of Pool's queue
                for t in ts:
                    nc.gpsimd.tensor_mul(hc_t[(t, li)][:, 0:H],
                                         sio[t][:, H:1024], tc_[t][:])
                    nc.sync.dma_start(
                        o_d[t * P:(t + 1) * P, li * 2 * H:(li + 1) * 2 * H],
                        hc_t[(t, li)][:])
                    if li < 3:
                        h16 = h16p.tile([P, H], dt.bfloat16, tag="h16",
                                        name=f"h16_{t}_{li}")
                        h16_t[(t, li)] = h16
                        nc.vector.tensor_mul(h16[:], sio[t][:, H:1024],
                                             tc_[t][:])

    nc.compile()
    return nc


# ---------------------------------------------------------------- entry point

def _ensure_axon_hooks():
    """bass_utils' trace path imports antenv.axon_hooks, which some images
    lack; install a shim that drives NTFF profiling via libaxon_pjrt.so
    (mirrors the boot-side _ntff_profile_via_ctypes) or degrades to None."""
    try:
        import antenv.axon_hooks  # noqa: F401
        return
    except ImportError:
        pass
    import types
    import contextlib
    import ctypes

    def _build_hook():
        so = "/opt/axon/libaxon_pjrt.so"
        try:
            lib = ctypes.CDLL(so)
        except OSError:
            return None
        if not hasattr(lib, "axon_start_nrt_profile"):
            return None
        lib.axon_start_nrt_profile.argtypes = [
            ctypes.POINTER(ctypes.c_int64), ctypes.c_size_t]
        lib.axon_start_nrt_profile.restype = ctypes.c_int64
        lib.axon_stop_nrt_profile.argtypes = [ctypes.c_char_p]
        lib.axon_stop_nrt_profile.restype = ctypes.c_int64

        @contextlib.contextmanager
        def _hook(output_dir, device_ids):
            import jax
            jax.devices()
            if device_ids:
                ids = (ctypes.c_int64 * len(device_ids))(*device_ids)
                rc = lib.axon_start_nrt_profile(ids, len(device_ids))
            else:
                rc = lib.axon_start_nrt_profile(None, 0)
            if rc != 0:
                raise RuntimeError(f"axon_start_nrt_profile rc={rc}")
            try:
                yield
            finally:
                n = lib.axon_stop_nrt_profile(str(output_dir).encode())
                print(f"ntff profile: {n} file(s) written to {output_dir}",
                      file=sys.stderr)

        return _hook

    box = [None, False]

    def set_axon_ntff_profile_hook(h):
        box[0] = h
        box[1] = True

    def get_axon_ntff_profile_hook():
        if not box[1]:
            box[0] = _build_hook()
            box[1] = True
        return box[0]

    mod = types.ModuleType("antenv.axon_hooks")
    mod.set_axon_ntff_profile_hook = set_axon_ntff_profile_hook
    mod.get_axon_ntff_profile_hook = get_axon_ntff_profile_hook
    import antenv
    sys.modules["antenv.axon_hooks"] = mod
    antenv.axon_hooks = mod


_cache = {}


def kernel(**inputs):
    packed_x = np.asarray(inputs["packed_x"], np.float32)
    bs = np.asarray(inputs["batch_sizes"])

    key = bs.tobytes()
    if key not in _cache:
        plan = _make_plan(bs)
        nc = _build_nc(plan["T_tiles"])
        _cache[key] = (plan, nc)
    plan, nc = _cache[key]

    w, b = _pack_weights(inputs)
    T_tiles = plan["T_tiles"]
    starts = plan["starts"]

    in_maps = []
    for c in range(NCORES):
        xr = packed_x[starts[c]:starts[c + 1]]
        in_maps.append({"x": _pack_x(xr, T_tiles), "w": w, "b": b})

    from concourse.bass_utils import run_bass_kernel_spmd
    _ensure_axon_hooks()
    res = run_bass_kernel_spmd(nc, in_maps, core_ids=list(range(NCORES)))
    global last_result
    last_result = res

    # Reassemble: distinct-row tensor D [U, 8*512], then the reference's own
    # ragged gather D[s] expands duplicates (pure indexing, no math).
    s = plan["s"]
    Dfull = np.empty((plan["U"], J * H), np.float32)
    for c in range(NCORES):
        n = plan["counts"][c]
        Dfull[starts[c]:starts[c + 1]] = np.asarray(res.results[c]["hc"])[:n]
    gathered = Dfull[s]                                   # [Nout, 8*512]
    full = {nm: np.ascontiguousarray(gathered[:, jo * H:(jo + 1) * H])
            for jo, nm in enumerate(OUT_NAMES)}

    return (full["h4"], full["h1"], full["c1"], full["h2"], full["c2"],
            full["h3"], full["c3"], full["h4"], full["c4"])


if __name__ == "__main__":
    import reference
    inputs = reference.setup_inputs()
    out = kernel(**{k: np.asarray(v) for k, v in inputs.items()})
    print([o.shape for o in out])
